# revision 1
# baseline (speedup 1.0000x reference)
"""Trainium2 Bass kernel for nn_CoLL_78065325572576 (moe_routing).

Reference computation (per voxel v of x[B,H,W,C], nb=8 bins):
    b_v   = floor(8*x_v)                       (bin index)
    temp  = co[i, b_v] * x_v                   (8 channels)
    conv  = depthwise 3x3x3 conv over (H,W,C)  (SAME pad, 8 channels)
    out_v = conv[v, b_v] + bias[b_v]

Kernel formulation used here (all equalities exact):
    s_q[v]  = x_v * 1[b_v == q]                 (mask-routed fields)
    out_v   = sum_p 1[b_v==p] * ( sum_{dv,q} K[dv,p]*co[p,q] * s_q[v+dv] + bias[p] )

Device mapping (per core, pure data-parallel over 8 cores = batch x W-half):
  - x replicated by DMA into partitions (q=8, hs=16); h tiled in 10
    overlapping windows of 16 rows (stride 14, valid interior 14).
  - ROUTE (custom DVE op): s = x masked per bin-group, bf16.
  - CONV: 9 accumulating banded bf16 matmuls on TensorE; lhsT
    [(q,hs),(p,hs')] = K[hs-hs'+1, dw+1, dc+1, p]*co[p,q] folds the
    8x8 channel mix and the dh taps; (dw,dc) are free-dim AP shifts.
  - SELECT (custom DVE op): masked = (x in bin p) ? g + bias[p] : 0.
  - REDUCE: ones-banded float32r matmul sums the 8 p-groups.
  - ScalarE drains PSUM; DMA writes the output shard.
"""

import numpy as np
import ml_dtypes

NB = 8
B, H, W, C = 4, 128, 128, 64
WS = 64            # output W per core
WH = WS + 2        # input W incl. halo
WIN = 16           # h-window rows (one partition group)
VALID = 14         # valid output rows per window
NWIN = 10          # h windows (stride 14): covers h in [0,128)
NCORES = 8
NSTRIPES = 4
WSTR = WS // NSTRIPES        # 16 output w per stripe
WSTR_IN = WSTR + 2           # 18 input w per stripe
FSTR_IN = WSTR_IN * C        # 1152
CP = C + 2                   # c padded with zero border cols in routed tensor
FSTR_SL = WSTR_IN * CP       # 1188
TAPS = [(dw, dc) for dc in (0, -1, 1) for dw in (-1, 0, 1)]  # dc=0 first

_prog_cache: dict = {}


# --------------------------------------------------------------------------- #
# custom DVE ops (registered at import into concourse.dve_ops)                #
# --------------------------------------------------------------------------- #

def _register_ops():
    from concourse import dve_ops
    from concourse.dve_spec import (
        Spec, Src0, Src1, C0, C1, C2, Zero, lower, select, _has_src1,
    )
    from concourse.dve_uop import DveOpSpec

    if "ANT_ROUTE_BIN8" in dve_ops._SUB_OPCODE_FOR_NAME:
        ops = {op.name: op for op in dve_ops.OPS}
        return ops["ANT_ROUTE_BIN8"], ops["ANT_SEL_BIN8"]

    def reg(name, spec, subdim=False):
        row = dve_ops._CUSTOM_DVE_ROW_BASE + len(dve_ops.OPS)
        assert row < 0x20, "custom DVE opcode rows exhausted"
        dve_ops._SUB_OPCODE_FOR_NAME[name] = row
        shas = {}
        for ver in ("v3", "v4"):
            try:
                s = DveOpSpec(name=name, opcode=row,
                              uops=lower(spec, ver=ver),
                              rd1_en=_has_src1(spec))
                shas[ver] = s.sha(ver)
            except Exception:
                pass
        op = dve_ops.DveOp(name, spec, subdim=subdim, uops_sha=shas)
        dve_ops.OPS.append(op)
        dve_ops.CUSTOM_DVE_SPECS[name] = spec
        return op

    def _bc(v):
        return v if isinstance(v, float) else np.asarray(v).reshape(-1, 1)

    # s = x if (x >= lo) & (x < hi) else 0   (lo/hi per-partition scalars)
    route = reg("ANT_ROUTE_BIN8", Spec(
        body=select((Src0 >= C0) & (Src0 < C1), Src0, Zero),
        reference=lambda in0, in1, s0, s1, imm2: np.where(
            (in0 >= _bc(s0)) & (in0 < _bc(s1)), in0, 0.0).astype(np.float32),
    ))

    # masked = (x >= lo) & (x < lo + width) ? g + bias : 0
    #   in0 = g (PSUM), in1 = x (center voxel), s0 = lo, s1 = bias,
    #   imm2 = 1/8 bin width (compile-time literal)
    selb = reg("ANT_SEL_BIN8", Spec(
        body=select((Src1 >= C0) & (Src1 < (C0 + C2)), Src0 + C1, Zero),
        reference=lambda in0, in1, s0, s1, imm2: np.where(
            (in1 >= _bc(s0)) & (in1 < (_bc(s0) + imm2)),
            in0 + _bc(s1), 0.0).astype(np.float32),
    ))
    return route, selb


# --------------------------------------------------------------------------- #
# host-side constant construction                                             #
# --------------------------------------------------------------------------- #

def _band_lhsT(dw_kernel, co_matrix, dwi, dci):
    """lhsT[(q,hs),(p,hs')] = K[dh+1, dwi+1, dci+1, p] * co[p,q], dh=hs-hs',
    for hs' in [1,15), |dh| <= 1."""
    K = np.asarray(dw_kernel, np.float32)       # [3,3,3,1,8]
    co = np.asarray(co_matrix, np.float32)      # [8,8]
    lhsT = np.zeros((128, 128), np.float32)
    hsp = np.arange(1, 15)
    for q in range(NB):
        for p in range(NB):
            for dh in (-1, 0, 1):
                a = K[dh + 1, dwi + 1, dci + 1, 0, p] * co[p, q]
                lhsT[q * WIN + hsp + dh, p * WIN + hsp] = a
    return lhsT


def _make_consts(co_matrix, dw_kernel, dw_bias):
    conv_w = np.stack([_band_lhsT(dw_kernel, co_matrix, dwi, dci)
                       for (dwi, dci) in TAPS])          # [9,128,128]
    red_w = np.zeros((128, VALID), np.float32)
    for p in range(NB):
        for hspp in range(VALID):
            red_w[p * WIN + hspp + 1, hspp] = 1.0
    part = np.arange(128)
    bins_lo = ((part // WIN) / NB).astype(np.float32).reshape(128, 1)
    bias_p = np.asarray(dw_bias, np.float32)[part // WIN].reshape(128, 1)
    return {
        "conv_w": conv_w.astype(ml_dtypes.bfloat16),
        "red_w": red_w,
        "bins_lo": bins_lo,
        "bias_p": bias_p,
    }


def _shard(x, core):
    b, wh = core // 2, core % 2
    xp = np.zeros((H, WH, C), np.float32)
    lo, hi = wh * WS - 1, wh * WS + WS + 1
    slo, shi = max(0, lo), min(W, hi)
    xp[:, slo - lo:shi - lo, :] = x[b, :, slo:shi, :]
    return xp


# --------------------------------------------------------------------------- #
# device program                                                              #
# --------------------------------------------------------------------------- #

def _build_program(reps=1, with_bias=False):
    import concourse.mybir as mybir
    import concourse.tile as tile
    from concourse import bacc
    import bass_rust

    def raw_ap(base_ap, dims, offset):
        a = base_ap.copy()
        a.ap = bass_rust.VecI64Pair(dims)
        a.offset = offset
        return a

    ROUTE, SELB = _register_ops()
    f32 = mybir.dt.float32
    bf16 = mybir.dt.bfloat16
    f32r = mybir.dt.float32r

    nc = bacc.Bacc("TRN2", target_bir_lowering=False, debug=False)
    x_d = nc.dram_tensor("x_s", [H, WH, C], f32, kind="ExternalInput")
    cw_d = nc.dram_tensor("conv_w", [9, 128, 128], bf16, kind="ExternalInput")
    rw_d = nc.dram_tensor("red_w", [128, VALID], f32r, kind="ExternalInput")
    lo_d = nc.dram_tensor("bins_lo", [128, 1], f32, kind="ExternalInput")
    bi_d = nc.dram_tensor("bias_p", [128, 1], f32, kind="ExternalInput")
    out_d = nc.dram_tensor("out_s", [H, WS, C], f32, kind="ExternalOutput")

    with tile.TileContext(nc) as tc:
        with (
            tc.tile_pool(name="const", bufs=1) as cpool,
            tc.tile_pool(name="xr", bufs=2) as xrpool,
            tc.tile_pool(name="sl", bufs=2) as slpool,
            tc.tile_pool(name="mk", bufs=2) as mkpool,
            tc.tile_pool(name="msk", bufs=2) as mskpool,
            tc.tile_pool(name="ost", bufs=3) as ostpool,
            tc.tile_pool(name="ps", bufs=3, space="PSUM") as pspool,
            tc.tile_pool(name="ps2", bufs=3, space="PSUM") as ps2pool,
        ):
            cw = cpool.tile([128, 9 * 128], bf16)
            nc.sync.dma_start(
                cw[:, :],
                raw_ap(cw_d[0], [[128, 128], [128 * 128, 9], [1, 128]], 0))
            rw = cpool.tile([128, VALID], f32r)
            nc.sync.dma_start(rw[:], rw_d[:])
            lo = cpool.tile([128, 1], f32)
            nc.sync.dma_start(lo[:], lo_d[:])
            bi = cpool.tile([128, 1], f32)
            nc.sync.dma_start(bi[:], bi_d[:])
            hi = cpool.tile([128, 1], f32)
            nc.vector.tensor_scalar_add(hi[:], lo[:], 1.0 / NB)

            for rep in range(reps):
              for st in range(NSTRIPES):
                  # ---- load x replicated into (q, hs) x (hw, w, c) ---------- #
                  xr0 = xrpool.tile([128, 1, FSTR_IN], f32, tag="xr0")
                  xr = xrpool.tile([128, NWIN, FSTR_IN], f32, tag="xr")
                  # zero rows whose h falls outside [0, 128)
                  nc.gpsimd.memset(xr0[:, 0, :], 0.0)
                  nc.gpsimd.memset(xr[:, 9, :], 0.0)
                  wb = st * WSTR
                  for q in range(NB):
                      nc.sync.dma_start(
                          xr0[q * WIN + 1:(q + 1) * WIN, 0, :],
                          x_d[0:15, wb:wb + WSTR_IN, :])
                  for q in range(NB):
                      nc.sync.dma_start(
                          xr[q * WIN:(q + 1) * WIN, 1:9, :],
                          raw_ap(x_d[0:WIN, 0:WSTR_IN, :],
                                 [[WH * C, WIN], [14 * WH * C, 8],
                                  [1, FSTR_IN]],
                                 13 * WH * C + wb * C))
                      nc.sync.dma_start(
                          xr[q * WIN:q * WIN + 3, 9, :],
                          x_d[125:128, wb:wb + WSTR_IN, :])

                  # ---- route to bf16 bin fields (c padded to 66) ------------ #
                  sl = slpool.tile([128, NWIN, WSTR_IN, CP], bf16, tag="sl")
                  # zero the c-border columns once per stripe
                  nc.gpsimd.memset(sl[:, :, :, 0], 0.0)
                  nc.gpsimd.memset(sl[:, :, :, CP - 1], 0.0)
                  for hw in range(NWIN):
                      xsrc = xr0[:, 0, :] if hw == 0 else xr[:, hw, :]
                      nc.vector._custom_dve(
                          ROUTE, out=sl[:, hw, :, 1:1 + C],
                          in0=xsrc.rearrange("p (w c) -> p w c", c=C),
                          s0=lo[:], s1=hi[:])

                  # ---- conv + select + reduce per 512-col chunk ------------- #
                  # Tail stages (select/reduce/drain) are emitted one chunk
                  # behind the conv matmuls so PE never waits on the DVE
                  # select of the chunk it just accumulated.
                  pend = []

                  def flush_tail(pend=pend, st=st):
                      if not pend:
                          return
                      ps, hw, wc = pend.pop(0)
                      mk = mkpool.tile([128, 512], f32r, tag="mk",
                                       name=f"mk_{st}_{hw}_{wc}")
                      xcsrc = xr0 if hw == 0 else xr
                      xcen = xcsrc[:, hw if hw else 0,
                                   (wc * 8 + 1) * C:(wc * 8 + 9) * C]
                      nc.vector._custom_dve(
                          SELB, out=mk[:], in0=ps[:],
                          in1=xcen, s0=lo[:], s1=bi[:], imm2=1.0 / NB)

                      p2 = ps2pool.tile([VALID, 512], f32, tag="p2",
                                        name=f"p2_{st}_{hw}_{wc}")
                      nc.tensor.matmul(p2[:], rw[:], mk[:],
                                       start=True, stop=True)

                      ost = osts[hw]
                      nc.scalar.copy(ost[:, wc * 512:(wc + 1) * 512], p2[:])
                      if wc == WSTR // 8 - 1:
                          rows = VALID if hw < 9 else 2
                          dst = out_d[14 * hw:14 * hw + rows,
                                      st * WSTR:(st + 1) * WSTR, :]
                          nc.gpsimd.dma_start(dst, ost[0:rows, :])

                  osts = {}
                  for hw in range(NWIN):
                      osts[hw] = ostpool.tile([VALID, WSTR * C], f32,
                                              tag="ost",
                                              name=f"ost_{st}_{hw}")
                      for wc in range(WSTR // 8):
                          ps = pspool.tile([128, 512], f32, tag="ps",
                                           name=f"ps_{st}_{hw}_{wc}")
                          for ti, (dwi, dci) in enumerate(TAPS):
                              w0 = wc * 8 + 1 + dwi
                              rhs = sl[:, hw, w0:w0 + 8, 1 + dci:1 + dci + C]
                              nc.tensor.matmul(
                                  ps[:], cw[:, ti * 128:(ti + 1) * 128], rhs,
                                  start=(ti == 0), stop=(ti == len(TAPS) - 1))
                          pend.append((ps, hw, wc))
                          if len(pend) > 1:
                              flush_tail()
                  while pend:
                      flush_tail()

    nc.compile()
    return nc


def _get_program(reps=1, with_bias=False):
    key = (reps, with_bias)
    if key not in _prog_cache:
        _prog_cache[key] = _build_program(reps, with_bias)
    return _prog_cache[key]


# --------------------------------------------------------------------------- #
# entry point                                                                 #
# --------------------------------------------------------------------------- #

def kernel(x, co_matrix, dw_kernel, dw_bias):
    from concourse.bass_utils import run_bass_kernel_spmd

    x = np.asarray(x, np.float32)
    consts = _make_consts(co_matrix, dw_kernel, dw_bias)
    nc = _get_program(with_bias=bool(np.any(np.asarray(dw_bias))))

    in_maps = []
    for core in range(NCORES):
        m = {"x_s": _shard(x, core)}
        m.update(consts)
        in_maps.append(m)

    res = run_bass_kernel_spmd(nc, in_maps, core_ids=list(range(NCORES)))
    out = np.zeros((B, H, W, C), np.float32)
    for core in range(NCORES):
        b, wh = core // 2, core % 2
        out[b, :, wh * WS:(wh + 1) * WS, :] = res.results[core]["out_s"]
    return out



# revision 6
# speedup vs baseline: 3.0994x; 3.0994x over previous
"""Trainium2 Bass kernel for nn_CoLL_78065325572576 (moe_routing).

Reference computation (per voxel v of x[B,H,W,C], nb=8 bins):
    b_v   = floor(8*x_v)                       (bin index)
    temp  = co[i, b_v] * x_v                   (8 channels)
    conv  = depthwise 3x3x3 conv over (H,W,C)  (SAME pad, 8 channels)
    out_v = conv[v, b_v] + bias[b_v]

Kernel formulation (all vector-engine builtins; no custom DVE ops, no
gpsimd, no matmuls, minimal DMA count — instruction/queue overhead
dominates the runtime here, vector-engine ops are effectively free):

  Sharding: core = (batch, h-half). Per core volume [64,128,64].
  Host pads x to [66,130,66] per core (h/w/c halos + zero borders), so
  the device program has no edge cases and no memsets.

  SBUF layout: partition = (p=8 bins x j=16 w-blocks); each partition
  holds a haloed block [34h x 10w x 66c] flattened to 22440 (strides
  660/66/1), so all 27 conv taps are pure free-dim AP shifts.

  Per tile (2 per core, h-halves):
    1 DMA in (stride-0 partition replication of x into the 8 p-groups)
    w      = sum_q (co[p,q]-co[p,q-1]) * 1[x >= q/8]   (stairs -> co[p,bin])
    temp   = w * x                 (bf16, in place)
    msel   = 1[x>=hi_p] - 1[x>=lo_p]        (= -1 in own bin)
    acc    = sum_taps (-K[tap,p]) * temp_shifted        (f32, reuses x slot)
    sel    = (acc - bias_p) * msel                      (= (G+bias)*mask)
    out    = sum over p groups (3-op partition tree add)
    1 DMA out
"""

import numpy as np

NB = 8
B, H, W, C = 4, 128, 128, 64
NCORES = 8
HS = 34          # tile rows incl. h halo
WB = 10          # w cols per block incl. halo
CP = C + 2       # c padded
ROW = WB * CP    # 660
FLAT = HS * ROW  # 22440
CTR0 = ROW + CP + 1          # 727  first interior cell
CTR1 = 32 * ROW + 8 * CP + C  # 21712+1 -> use exclusive 21713
TAPS27 = [(dh, dw, dc) for dh in (-1, 0, 1) for dw in (-1, 0, 1)
          for dc in (-1, 0, 1)]
NCOLS = 27 + 8 + 3   # K taps | dco stairs | lo, hi, -bias

_prog_cache: dict = {}


# --------------------------------------------------------------------------- #
# host-side constant construction                                             #
# --------------------------------------------------------------------------- #

def _make_consts(co_matrix, dw_kernel, dw_bias):
    co = np.asarray(co_matrix, np.float32)
    K = np.asarray(dw_kernel, np.float32)
    bias = np.asarray(dw_bias, np.float32)
    p = np.arange(128) // 16
    ct = np.zeros((128, NCOLS), np.float32)
    for ti, (dh, dw, dc) in enumerate(TAPS27):
        ct[:, ti] = -K[dh + 1, dw + 1, dc + 1, 0, p]
    dco = co.copy()
    dco[:, 1:] = co[:, 1:] - co[:, :-1]
    for q in range(NB):
        ct[:, 27 + q] = dco[p, q]
    ct[:, 35] = p / 8.0
    ct[:, 36] = (p + 1) / 8.0
    ct[:, 37] = -bias[p]
    return {"ct": ct}


def _shard(x, core):
    b, hh = core // 2, core % 2
    xp = np.zeros((66, 130, 66), np.float32)
    h_lo = hh * 64 - 1
    g0, g1 = max(0, h_lo), min(H, h_lo + 66)
    xp[g0 - h_lo:g1 - h_lo, 1:129, 1:65] = x[b, g0:g1]
    # pre-blocked + p-replicated device layout: [66 h, 128 (p x j), 660]
    xj = np.stack([xp[:, 8 * j:8 * j + WB, :].reshape(66, ROW)
                   for j in range(16)], axis=1)          # [66, 16, 660]
    xb = np.broadcast_to(xj[:, None], (66, NB, 16, ROW))
    return np.ascontiguousarray(xb).reshape(66, 128, ROW)


# --------------------------------------------------------------------------- #
# device program                                                              #
# --------------------------------------------------------------------------- #

def _build_program(reps=1, with_bias=False):
    import concourse.mybir as mybir
    import concourse.tile as tile
    from concourse import bacc
    import bass_rust

    def raw_ap(base_ap, dims, offset):
        a = base_ap.copy()
        a.ap = bass_rust.VecI64Pair(dims)
        a.offset = offset
        return a

    f32 = mybir.dt.float32
    bf16 = mybir.dt.bfloat16
    Op = mybir.AluOpType

    nc = bacc.Bacc("TRN2", target_bir_lowering=False, debug=False)
    x_d = nc.dram_tensor("x_s", [66, 128, ROW], f32, kind="ExternalInput")
    ct_d = nc.dram_tensor("ct", [128, NCOLS], f32, kind="ExternalInput")
    out_d = nc.dram_tensor("out_s", [2, 128, 16384], f32,
                            kind="ExternalOutput")

    CTR = slice(727, 21713)

    with tile.TileContext(nc) as tc:
        with (
            tc.tile_pool(name="const", bufs=1) as cpool,
            tc.tile_pool(name="a", bufs=1) as pa,
            tc.tile_pool(name="w", bufs=1) as pw,
            tc.tile_pool(name="m", bufs=1) as pm,
        ):
            ct = cpool.tile([128, NCOLS], f32)
            nc.sync.dma_start(ct[:, :], ct_d[:, :])

            def col(i):
                return ct[:, i:i + 1]

            for rep in range(reps):
                for t in range(2):
                    h0 = 32 * t
                    a = pa.tile([128, HS, WB, CP], f32, tag="a",
                                name=f"a_{rep}_{t}")
                    nc.sync.dma_start(
                        a[:, :, :, :],
                        raw_ap(x_d[0],
                               [[ROW, 128], [128 * ROW, HS], [1, ROW]],
                               h0 * 128 * ROW))
                    af = a.rearrange("p h w c -> p (h w c)")
                    w = pw.tile([128, FLAT], bf16, tag="w", name=f"w_{rep}_{t}")
                    m = pm.tile([128, FLAT], bf16, tag="m", name=f"m_{rep}_{t}")
                    # w = sum_q dco_q * 1[x >= q/8]  (stairs -> co[p, bin])
                    nc.vector.tensor_scalar(w[:, :], af[:, :], 0.0, col(27),
                                            op0=Op.is_ge, op1=Op.mult)
                    for q in range(1, NB):
                        nc.vector.tensor_scalar(m[:, :], af[:, :], q / 8.0,
                                                col(27 + q),
                                                op0=Op.is_ge, op1=Op.mult)
                        nc.vector.tensor_add(w[:, :], w[:, :], m[:, :])
                    # temp = w * x (bf16, in place over w)
                    nc.vector.tensor_mul(w[:, :], w[:, :], af[:, :])
                    # msel = 1[x>=hi] - 1[x>=lo]  (-1 in own bin)
                    nc.vector.tensor_scalar(m[:, CTR], af[:, CTR], col(35),
                                            None, op0=Op.is_ge)
                    nc.vector.scalar_tensor_tensor(
                        m[:, CTR], af[:, CTR], col(36), m[:, CTR],
                        op0=Op.is_ge, op1=Op.subtract)
                    # conv: acc = sum_taps (-K)*temp_shift, into the x slot
                    for ti, (dh, dw_, dc) in enumerate(TAPS27):
                        d = dh * ROW + dw_ * CP + dc
                        src = w[:, 727 + d:21713 + d]
                        if ti == 0:
                            nc.vector.tensor_scalar(af[:, CTR], src, col(0),
                                                    None, op0=Op.mult)
                        else:
                            nc.vector.scalar_tensor_tensor(
                                af[:, CTR], src, col(ti), af[:, CTR],
                                op0=Op.mult, op1=Op.add)
                    # select + bias: af = (acc + (-bias)) * msel
                    nc.vector.scalar_tensor_tensor(
                        af[:, CTR], af[:, CTR], col(37), m[:, CTR],
                        op0=Op.add, op1=Op.mult)
                    # compact per-p partials (p-sum happens on the host:
                    # cross-partition adds are illegal on the vector engine)
                    mc = pm.tile([128, 32, 8, C], f32, tag="m",
                                 name=f"mc_{rep}_{t}")
                    nc.vector.tensor_copy(mc[:, :, :, :],
                                          a[:, 1:33, 1:9, 1:65])
                    nc.sync.dma_start(
                        raw_ap(out_d[0],
                               [[16384, 128], [1, 16384]],
                               t * 128 * 16384),
                        mc[:, :, :, :])
    nc.compile()
    return nc


def _get_program(reps=1, with_bias=False):
    key = (reps, with_bias)
    if key not in _prog_cache:
        _prog_cache[key] = _build_program(reps, with_bias)
    return _prog_cache[key]


# --------------------------------------------------------------------------- #
# entry point                                                                 #
# --------------------------------------------------------------------------- #

def kernel(x, co_matrix, dw_kernel, dw_bias):
    from concourse.bass_utils import run_bass_kernel_spmd

    x = np.asarray(x, np.float32)
    consts = _make_consts(co_matrix, dw_kernel, dw_bias)
    nc = _get_program()

    in_maps = []
    for core in range(NCORES):
        m = {"x_s": _shard(x, core)}
        m.update(consts)
        in_maps.append(m)

    res = run_bass_kernel_spmd(nc, in_maps, core_ids=list(range(NCORES)))
    out = np.zeros((B, H, W, C), np.float32)
    for core in range(NCORES):
        b, hh = core // 2, core % 2
        r = res.results[core]["out_s"].reshape(2, NB, 16, 32, 8, C)
        r = r.sum(axis=1)                       # p-sum -> [2, 16j, 32h, 8w, C]
        r = r.transpose(0, 2, 1, 3, 4).reshape(64, W, C)
        out[b, hh * 64:(hh + 1) * 64, :, :] = r
    return out


# revision 7
# speedup vs baseline: 7.6365x; 2.4639x over previous
"""Trainium2 Bass kernel for nn_CoLL_78065325572576 (moe_routing).

Reference computation (per voxel v of x[B,H,W,C], nb=8 bins):
    b_v   = floor(8*x_v)                       (bin index)
    temp  = co[i, b_v] * x_v                   (8 channels)
    conv  = depthwise 3x3x3 conv over (H,W,C)  (SAME pad, 8 channels)
    out_v = conv[v, b_v] + bias[b_v]

The measured cost on this runtime is dominated by HBM traffic (~1ms/MB
marginal) plus large fixed costs for custom-DVE/gpsimd/scalar-engine
instructions; vector-engine builtins are effectively free. So the kernel
minimizes HBM bytes and uses only vector builtins:

  Sharding: core = (batch, h-half). Per core volume [64,128,64].
  Partition = 128 spatial blocks (16 w-blocks x 8 h-blocks); each
  partition holds a haloed block [10h x 10w x 66c] = 6600 f32 (host
  pre-blocks x, zero borders included, so no device memsets/edge cases
  and all 27 conv taps are free-dim AP shifts).

  Per core: 1 input DMA (3.4MB), one pass per bin channel p (all
  immediate-scalar vector builtins):
    w_p    = sum_q (co[p,q]-co[p,q-1]) * 1[x >= q/8]    (-> co[p,bin])
    temp_p = w_p * x                                    (bf16)
    acc_p  = sum_taps (-K[tap,p]) * temp_p-shifted      (f32)
    msel_p = 1[x>=(p+1)/8] - 1[x>=p/8]                  (= -1 in own bin)
    acc_p  = (acc_p - bias[p]) * msel_p                 (= (G+bias)*mask)
    out   += acc_p interior                             (compact)
  then 1 output DMA (2.1MB). The 8-channel overcompute lives entirely in
  free vector ops instead of replicated HBM traffic.
"""

import numpy as np

NB = 8
B, H, W, C = 4, 128, 128, 64
NCORES = 8
BK = 8            # block edge (h and w)
HB = BK + 2       # haloed block edge
CP = C + 2        # c padded
ROW = HB * CP     # 660
FLAT = HB * ROW   # 6600
INT0 = ROW + CP + 1            # 727: first interior cell (h=1,w=1,c=1)
INT1 = 8 * ROW + 8 * CP + 64   # 5872: last interior cell (h=8,w=8,c=64)
TAPS27 = [(dh, dw, dc) for dh in (-1, 0, 1) for dw in (-1, 0, 1)
          for dc in (-1, 0, 1)]

_prog_cache: dict = {}


# --------------------------------------------------------------------------- #
# host-side sharding                                                          #
# --------------------------------------------------------------------------- #

def _shard(x, core):
    """Pre-blocked per-core input: [128 (j16 x hb8), 6600] f32."""
    b, hh = core // 2, core % 2
    xp = np.zeros((66, 130, 66), np.float32)
    h_lo = hh * 64 - 1
    g0, g1 = max(0, h_lo), min(H, h_lo + 66)
    xp[g0 - h_lo:g1 - h_lo, 1:129, 1:65] = x[b, g0:g1]
    xb = np.empty((16, 8, FLAT), np.float32)
    for j in range(16):
        for hb in range(8):
            xb[j, hb] = xp[8 * hb:8 * hb + HB,
                           8 * j:8 * j + HB, :].reshape(FLAT)
    return xb.reshape(128, FLAT)


def _unshard(partials):
    """[8 cores][128, 4096] -> [B,H,W,C]."""
    out = np.zeros((B, H, W, C), np.float32)
    for core in range(NCORES):
        b, hh = core // 2, core % 2
        r = partials[core].reshape(16, 8, BK, BK, C)   # j, hb, h, w, c
        r = r.transpose(1, 2, 0, 3, 4).reshape(64, W, C)
        out[b, hh * 64:(hh + 1) * 64] = r
    return out


# --------------------------------------------------------------------------- #
# device program                                                              #
# --------------------------------------------------------------------------- #

def _build_program(co_matrix, dw_kernel, dw_bias, reps=1):
    import concourse.mybir as mybir
    import concourse.tile as tile
    from concourse import bacc

    co = np.asarray(co_matrix, np.float64)
    K = np.asarray(dw_kernel, np.float64)
    bias = np.asarray(dw_bias, np.float64)
    dco = co.copy()
    dco[:, 1:] = co[:, 1:] - co[:, :-1]

    f32 = mybir.dt.float32
    bf16 = mybir.dt.bfloat16
    Op = mybir.AluOpType

    nc = bacc.Bacc("TRN2", target_bir_lowering=False, debug=False)
    x_d = nc.dram_tensor("x_s", [128, FLAT], f32, kind="ExternalInput")
    out_d = nc.dram_tensor("out_s", [128, BK * BK * C], f32,
                           kind="ExternalOutput")

    CTR = slice(INT0, INT1 + 1)
    L = INT1 + 1 - INT0

    with tile.TileContext(nc) as tc:
        with (
            tc.tile_pool(name="x", bufs=1) as px,
            tc.tile_pool(name="w", bufs=1) as pw,
            tc.tile_pool(name="u", bufs=1) as pu,
            tc.tile_pool(name="acc", bufs=1) as pacc,
            tc.tile_pool(name="msk", bufs=1) as pmsk,
            tc.tile_pool(name="o", bufs=1) as po,
        ):
            for rep in range(reps):
                xt = px.tile([128, HB, HB, CP], f32, tag="x",
                             name=f"x_{rep}")
                nc.sync.dma_start(
                    xt.rearrange("p h w c -> p (h w c)")[:, :], x_d[:, :])
                xf = xt.rearrange("p h w c -> p (h w c)")
                ot = po.tile([128, BK, BK, C], f32, tag="o", name=f"o_{rep}")
                for p in range(NB):
                    sfx = f"_{rep}_{p}"
                    w = pw.tile([128, FLAT], bf16, tag="w", name="w" + sfx)
                    u = pu.tile([128, FLAT], bf16, tag="u", name="u" + sfx)
                    # w = co[p, bin(x)] via stairs of is_ge steps
                    nc.vector.tensor_scalar(w[:, :], xf[:, :], 0.0,
                                            float(dco[p, 0]),
                                            op0=Op.is_ge, op1=Op.mult)
                    for q in range(1, NB):
                        nc.vector.tensor_scalar(u[:, :], xf[:, :], q / 8.0,
                                                float(dco[p, q]),
                                                op0=Op.is_ge, op1=Op.mult)
                        nc.vector.tensor_add(w[:, :], w[:, :], u[:, :])
                    # temp_p = w * x (bf16, in place)
                    nc.vector.tensor_mul(w[:, :], w[:, :], xf[:, :])
                    # conv: acc = sum_taps (-K[tap,p]) * temp_shift
                    acc = pacc.tile([128, HB, HB, CP], f32, tag="acc",
                                    name="acc" + sfx)
                    accf = acc.rearrange("p h w c -> p (h w c)")
                    for ti, (dh, dw_, dc) in enumerate(TAPS27):
                        d = dh * ROW + dw_ * CP + dc
                        src = w[:, INT0 + d:INT1 + 1 + d]
                        kv = float(-K[dh + 1, dw_ + 1, dc + 1, 0, p])
                        if ti == 0:
                            nc.vector.tensor_scalar(accf[:, CTR], src, kv,
                                                    None, op0=Op.mult)
                        else:
                            nc.vector.scalar_tensor_tensor(
                                accf[:, CTR], src, kv, accf[:, CTR],
                                op0=Op.mult, op1=Op.add)
                    # msel = 1[x>=hi] - 1[x>=lo]  (= -1 in own bin)
                    msk = pmsk.tile([128, L], bf16, tag="m", name="m" + sfx)
                    nc.vector.tensor_scalar(msk[:, :], xf[:, CTR], p / 8.0,
                                            None, op0=Op.is_ge)
                    nc.vector.scalar_tensor_tensor(
                        msk[:, :], xf[:, CTR], (p + 1) / 8.0, msk[:, :],
                        op0=Op.is_ge, op1=Op.subtract)
                    # select + bias (in place on acc)
                    nc.vector.scalar_tensor_tensor(
                        accf[:, CTR], accf[:, CTR], float(-bias[p]),
                        msk[:, :], op0=Op.add, op1=Op.mult)
                    # out (+)= selected interior, compacted
                    ai = acc[:, 1:1 + BK, 1:1 + BK, 1:1 + C]
                    if p == 0:
                        nc.vector.tensor_copy(ot[:, :, :, :], ai)
                    else:
                        nc.vector.tensor_add(ot[:, :, :, :], ot[:, :, :, :],
                                             ai)
                nc.sync.dma_start(
                    out_d[:, :],
                    ot.rearrange("p h w c -> p (h w c)")[:, :])
    nc.compile()
    return nc


def _get_program(co_matrix, dw_kernel, dw_bias, reps=1):
    key = (reps, float(np.asarray(co_matrix).sum()),
           float(np.asarray(dw_kernel).sum()))
    if key not in _prog_cache:
        _prog_cache[key] = _build_program(co_matrix, dw_kernel, dw_bias, reps)
    return _prog_cache[key]


# --------------------------------------------------------------------------- #
# entry point                                                                 #
# --------------------------------------------------------------------------- #

def kernel(x, co_matrix, dw_kernel, dw_bias):
    from concourse.bass_utils import run_bass_kernel_spmd

    x = np.asarray(x, np.float32)
    nc = _get_program(co_matrix, dw_kernel, dw_bias)

    in_maps = [{"x_s": _shard(x, core)} for core in range(NCORES)]
    res = run_bass_kernel_spmd(nc, in_maps, core_ids=list(range(NCORES)))
    return _unshard([res.results[c]["out_s"] for c in range(NCORES)])
